# revision 4
# baseline (speedup 1.0000x reference)
"""Trainium2 kernel for nn_CompositeOneGRU (gnn_message_passing).

Math notes (derived from the reference):
  - Only row 0 of each sample's GCN state feeds the output heads
    (x1 = relu(new_mem)[0]), and `proposed` depends only on the current
    sample's features, so the per-sample message passing collapses to a
    [6,32] scatter/degree matrix G_b per sample with
        proposed_b[0,:] = concat_r(G_b[r] @ x_b) @ W_flat,
    W_flat = vstack(conv_W[0..4], W0)  (all index work is host preprocessing).
  - update_gate == 1 makes the scan carry vacuous; the general case is
    handled exactly on the host (it is linear pre-relu).
  - Device work (the heavy part): X1^T = relu(W_flat^T @ Y^T)  then the two
    vocab heads, sharded 8-way over the vocabulary (6250 glob + 2500 sense
    columns per core), with log_softmax denominators combined via a tiny
    AllReduce:  out = Ln(exp(logit) * (1/S_global)).
"""

import sys

sys.path.insert(0, "/opt/trn_rl_repo")

import numpy as np
import ml_dtypes

import concourse.bacc as bacc
import concourse.bass as bass
import concourse.tile as tile
import concourse.mybir as mybir
from concourse.bass_utils import run_bass_kernel_spmd

B, N, D, R, E = 512, 32, 300, 5, 256
VG, VS = 50000, 20000
NCORES = 8
VG_SH, VS_SH = VG // NCORES, VS // NCORES          # 6250 / 2500 per core
NTILE = 512
NT_G, NT_S = 13, 5                                  # 512-wide col tiles (padded)
GW, SW = NT_G * NTILE, NT_S * NTILE                 # 6656 / 2560
OUTW = GW + SW                                      # 9216
NT = NT_G + NT_S                                    # 18
K = 6 * D                                           # 1800
KC, NKC = 120, 15                                   # K chunking for stage A
PAD_BIAS = -60.0                                    # padded logit; exp() ~ 1e-26

f32 = mybir.dt.float32
f16 = mybir.dt.float16
AF = mybir.ActivationFunctionType

_CACHE = {}


def _build_device():
    nc = bacc.Bacc("TRN2", target_bir_lowering=False, debug=False,
                   num_devices=NCORES)

    yT = nc.dram_tensor("yT", [K, B], f16, kind="ExternalInput")
    wflat = nc.dram_tensor("wflat", [K, D], f16, kind="ExternalInput")
    whead = nc.dram_tensor("whead", [NT, D + 1, NTILE], f16, kind="ExternalInput")
    ones_d = nc.dram_tensor("ones", [1, B], f16, kind="ExternalInput")
    out = nc.dram_tensor("out", [B, OUTW], f16, kind="ExternalOutput")

    DT = [(0, 128), (128, 256), (256, 300)]          # d-tiles of stage A
    NMT = B // 128                                   # 4 sample tiles

    with tile.TileContext(nc) as tc:
        with (
            tc.tile_pool(name="sba", bufs=1) as sba,        # stage A resident
            tc.tile_pool(name="sbw", bufs=3) as sbw,        # head weight stream
            tc.tile_pool(name="sbe", bufs=1) as sbe,        # exp values, resident
            tc.tile_pool(name="sbo", bufs=6) as sbo,        # out staging
            tc.tile_pool(name="psa", bufs=2, space="PSUM") as psa,
            tc.tile_pool(name="psh", bufs=4, space="PSUM") as psh,
            tc.tile_pool(name="dram", bufs=1, space="DRAM") as dram,
        ):
            # ---------------- stage A: X1^T = relu(W_flat^T @ Y^T) ----------
            yts, wfs = [], []
            for kc in range(NKC):
                yt = sba.tile([KC, B], f16, name=f"yt{kc}")
                wf = sba.tile([KC, D], f16, name=f"wf{kc}")
                nc.sync.dma_start(out=yt[:], in_=yT[kc * KC:(kc + 1) * KC, :])
                nc.sync.dma_start(out=wf[:], in_=wflat[kc * KC:(kc + 1) * KC, :])
                yts.append(yt)
                wfs.append(wf)

            x1 = [
                sba.tile([128, B], f16, name="x1a"),
                sba.tile([128, B], f16, name="x1b"),
                sba.tile([45, B], f16, name="x1c"),   # 44 rows + ones row
            ]
            for dt, (d0, d1) in enumerate(DT):
                dk = d1 - d0
                pp = psa.tile([dk, B], f32, tag="pp", name="pp")
                for kc in range(NKC):
                    nc.tensor.matmul(pp[:], lhsT=wfs[kc][:, d0:d1], rhs=yts[kc][:],
                                     start=(kc == 0), stop=(kc == NKC - 1))
                nc.scalar.activation(x1[dt][0:dk, :], pp[:], AF.Relu)
            # compute ops need quadrant-aligned partition starts; DMA doesn't
            nc.sync.dma_start(out=x1[2][44:45, :], in_=ones_d[:])

            # ---------------- stage B: heads + exp/accumulate ----------------
            KH = [(0, 128, 0), (128, 256, 1), (256, 301, 2)]  # whead rows -> x1 idx
            e_all = sbe.tile([128, NMT * OUTW], f32, name="e_all")
            s_all = sba.tile([128, NMT * NT], f32, name="s_all")
            s_pack = sba.tile([128, NMT * 2], f32, name="s_pack")
            r_all = sba.tile([128, NMT * 2], f32, name="r_all")

            for nt in range(NT):
                whs = []
                for i, (r0, r1, _) in enumerate(KH):
                    wh = sbw.tile([r1 - r0, NTILE], f16, tag=f"wh{i}", name=f"wh{i}")
                    nc.sync.dma_start(out=wh[:], in_=whead[nt, r0:r1, :])
                    whs.append(wh)
                for mt in range(NMT):
                    msl = slice(128 * mt, 128 * (mt + 1))
                    pt = psh.tile([128, NTILE], f32, tag="pt", name="pt")
                    for i, (r0, r1, xi) in enumerate(KH):
                        nc.tensor.matmul(pt[:], lhsT=x1[xi][:, msl], rhs=whs[i][:],
                                         start=(i == 0), stop=(i == 2))
                    ecol = mt * OUTW + nt * NTILE
                    scol = mt * NT + nt
                    nc.scalar.activation(
                        e_all[:, ecol:ecol + NTILE], pt[:], AF.Exp,
                        accum_out=s_all[:, scol:scol + 1])

            # ------------- denominators: reduce, AllReduce, recip ------------
            cc_ins, cc_outs = [], []
            for mt in range(NMT):
                ci = dram.tile([128, 2], f32, name=f"cc_in{mt}")
                co = dram.tile([128, 2], f32, addr_space="Shared", name=f"cc_out{mt}")
                cc_ins.append(ci)
                cc_outs.append(co)

            for mt in range(NMT):
                s0 = mt * NT
                nc.vector.reduce_sum(s_pack[:, 2 * mt:2 * mt + 1],
                                     s_all[:, s0:s0 + NT_G],
                                     axis=mybir.AxisListType.X)
                nc.vector.reduce_sum(s_pack[:, 2 * mt + 1:2 * mt + 2],
                                     s_all[:, s0 + NT_G:s0 + NT],
                                     axis=mybir.AxisListType.X)
                nc.sync.dma_start(out=cc_ins[mt][:], in_=s_pack[:, 2 * mt:2 * mt + 2])
                nc.gpsimd.collective_compute(
                    "AllReduce", mybir.AluOpType.add,
                    replica_groups=[list(range(NCORES))],
                    ins=[cc_ins[mt].opt()], outs=[cc_outs[mt].opt()])
                sg = sba.tile([128, 2], f32, tag="sg", name="sg", bufs=4)
                nc.sync.dma_start(out=sg[:], in_=cc_outs[mt][:])
                nc.vector.reciprocal(r_all[:, 2 * mt:2 * mt + 2], sg[:])

            # ---------------- out = Ln(exp * 1/S), streamed ------------------
            for mt in range(NMT):
                for nt in range(NT):
                    hcol = 2 * mt + (0 if nt < NT_G else 1)
                    ecol = mt * OUTW + nt * NTILE
                    o = sbo.tile([128, NTILE], f16, tag="o", name="o")
                    nc.scalar.activation(o[:], e_all[:, ecol:ecol + NTILE], AF.Ln,
                                         scale=r_all[:, hcol:hcol + 1])
                    nc.sync.dma_start(
                        out=out[128 * mt:128 * (mt + 1), nt * NTILE:(nt + 1) * NTILE],
                        in_=o[:])

    nc.compile()
    return nc


def _host_prep(x, edge_index, edge_type, conv_W, W0, update_gate,
               glob_W, glob_b, sense_W, sense_b, memory0):
    x = np.asarray(x, np.float32)
    ei = np.asarray(edge_index)
    et = np.asarray(edge_type).astype(np.int64)
    src, dst = ei[:, 0, :].astype(np.int64), ei[:, 1, :].astype(np.int64)

    bb = np.broadcast_to(np.arange(B)[:, None], (B, E))
    deg = np.ones((B, N, R), np.float32)
    np.add.at(deg, (bb, dst, et), 1.0)
    dinv = 1.0 / np.sqrt(deg)
    coeff = dinv[bb, src, et] * dinv[bb, dst, et]

    G = np.zeros((B, 6, N), np.float32)
    bm, em = np.nonzero(dst == 0)
    np.add.at(G, (bm, et[bm, em], src[bm, em]), coeff[bm, em])
    G[:, :R, 0] += dinv[:, 0, :] ** 2
    G[:, 5, 0] = 1.0

    Yf = np.einsum("bgn,bnd->bgd", G, x).reshape(B, K)
    Wflat = np.concatenate(
        [np.asarray(conv_W, np.float32).reshape(R * D, D),
         np.asarray(W0, np.float32)], axis=0)

    g = float(np.asarray(update_gate).reshape(-1)[0])
    mem0 = np.asarray(memory0, np.float32)
    if g == 1.0 and not np.any(mem0):
        yT_host = np.ascontiguousarray(Yf.T)
        wf_host = Wflat
    else:
        # exact host fallback for the general carry (linear pre-relu)
        P = Yf @ Wflat
        X1 = np.empty((B, D), np.float32)
        carry = mem0[0].copy()
        for b in range(B):
            carry = g * P[b] + (1.0 - g) * carry
            X1[b] = np.maximum(carry, 0.0)
        yT_host = np.zeros((K, B), np.float32)
        yT_host[:D] = X1.T
        wf_host = np.zeros((K, D), np.float32)
        wf_host[:D] = np.eye(D, dtype=np.float32)

    WcatT = np.concatenate(
        [np.asarray(glob_W, np.float32), np.asarray(sense_W, np.float32)], 0).T
    bcat = np.concatenate(
        [np.asarray(glob_b, np.float32), np.asarray(sense_b, np.float32)], 0)

    wheads = []
    for c in range(NCORES):
        blk = np.zeros((D + 1, OUTW), np.float32)
        blk[D, :] = PAD_BIAS
        g0 = VG_SH * c
        blk[:D, :VG_SH] = WcatT[:, g0:g0 + VG_SH]
        blk[D, :VG_SH] = bcat[g0:g0 + VG_SH]
        s0 = VG + VS_SH * c
        blk[:D, GW:GW + VS_SH] = WcatT[:, s0:s0 + VS_SH]
        blk[D, GW:GW + VS_SH] = bcat[s0:s0 + VS_SH]
        packed = np.ascontiguousarray(
            blk.reshape(D + 1, NT, NTILE).transpose(1, 0, 2)).astype(np.float16)
        wheads.append(packed)

    return yT_host.astype(np.float16), wf_host.astype(np.float16), wheads


def kernel(**inputs):
    if "nc" not in _CACHE:
        _CACHE["nc"] = _build_device()
    nc = _CACHE["nc"]

    yT_np, wf_np, wheads = _host_prep(**inputs)
    ones_np = np.ones((1, B), np.float16)
    in_maps = [{"yT": yT_np, "wflat": wf_np, "whead": wheads[c], "ones": ones_np}
               for c in range(NCORES)]

    import os
    trace = bool(int(os.environ.get("KERNEL_TRACE", "0")))
    res = run_bass_kernel_spmd(nc, in_maps, core_ids=list(range(NCORES)),
                               trace=trace)
    _CACHE["last_result"] = res

    outs = [res.results[c]["out"].astype(np.float32) for c in range(NCORES)]
    glob = np.concatenate([o[:, :VG_SH] for o in outs], axis=1)
    sense = np.concatenate([o[:, GW:GW + VS_SH] for o in outs], axis=1)
    return glob, sense


# revision 7
# speedup vs baseline: 1.1221x; 1.1221x over previous
"""Trainium2 kernel for nn_CompositeOneGRU (gnn_message_passing).

Math notes (derived from the reference):
  - Only row 0 of each sample's GCN state feeds the output heads
    (x1 = relu(new_mem)[0]), and `proposed` depends only on the current
    sample's features, so the per-sample message passing collapses to a
    [6,32] scatter/degree matrix G_b per sample with
        proposed_b[0,:] = concat_r(G_b[r] @ x_b) @ W_flat,
    W_flat = vstack(conv_W[0..4], W0)  (all index work is host preprocessing).
  - update_gate == 1 makes the scan carry vacuous; the general case is
    handled exactly on the host (it is linear pre-relu).
  - Device work (the heavy part): X1^T = relu(W_flat^T @ Y^T), then the two
    vocab heads sharded 8-way over the vocabulary (6250 glob + 2500 sense
    columns per core).  Raw logits stay in SBUF (f32); ScalarE computes
    exp with accumulated row-sums, one AllReduce per head combines the
    softmax denominators, and VectorE writes  out = logit - lse  as f16.
"""

import sys

sys.path.insert(0, "/opt/trn_rl_repo")

import numpy as np

import concourse.bacc as bacc
import concourse.bass as bass
import concourse.tile as tile
import concourse.mybir as mybir
from concourse.bass_utils import run_bass_kernel_spmd

B, N, D, R, E = 512, 32, 300, 5, 256
VG, VS = 50000, 20000
NCORES = 8
VG_SH, VS_SH = VG // NCORES, VS // NCORES      # 6250 / 2500 per core
W_SH = VG_SH + VS_SH                            # 8750 logit cols per core
NTILE = 350                                     # uniform n-tile (25 x 350)
NNT = W_SH // NTILE                             # 25
NGRP, GSZ = 5, 5                                # whead DMA groups: 5 x 5 tiles
K = 6 * D                                       # 1800
KC = 120                                        # stage-A k-chunk
NMT = B // 128                                  # 4 sample tiles
# head-respecting column ranges for exp / subtract / output staging
RANGES_G = [(0, 1750), (1750, 3500), (3500, 5250), (5250, 6250)]
RANGES_S = [(6250, 8000), (8000, 8750)]

f32 = mybir.dt.float32
f16 = mybir.dt.float16
AF = mybir.ActivationFunctionType

_CACHE = {}


def _build_device():
    nc = bacc.Bacc("TRN2", target_bir_lowering=False, debug=False,
                   num_devices=NCORES)

    yT = nc.dram_tensor("yT", [K, B], f16, kind="ExternalInput")
    wflat = nc.dram_tensor("wflat", [K, D], f16, kind="ExternalInput")
    wh_r0 = nc.dram_tensor("wh_r0", [NNT, 128, NTILE], f16, kind="ExternalInput")
    wh_r1 = nc.dram_tensor("wh_r1", [NNT, 128, NTILE], f16, kind="ExternalInput")
    wh_r2 = nc.dram_tensor("wh_r2", [NNT, 45, NTILE], f16, kind="ExternalInput")
    ones_d = nc.dram_tensor("ones", [1, B], f16, kind="ExternalInput")
    out = nc.dram_tensor("out", [B, W_SH], f16, kind="ExternalOutput")

    DT = [(0, 128), (128, 256), (256, 300)]

    with tile.TileContext(nc) as tc:
        with (
            tc.tile_pool(name="sba", bufs=1) as sba,
            tc.tile_pool(name="sbw", bufs=2) as sbw,
            tc.tile_pool(name="sbl", bufs=1) as sbl,
            tc.tile_pool(name="sbx", bufs=1) as sbx,
            tc.tile_pool(name="sbo", bufs=4) as sbo,
            tc.tile_pool(name="psa", bufs=2, space="PSUM") as psa,
            tc.tile_pool(name="psh", bufs=4, space="PSUM") as psh,
            tc.tile_pool(name="dram", bufs=1, space="DRAM") as dram,
        ):
            # ------------- stage A: X1^T = relu(W_flat^T @ Y^T) -------------
            ytc = sba.tile([KC, 15, B], f16, name="ytc")
            wfc = sba.tile([KC, 15, D], f16, name="wfc")
            nc.sync.dma_start(out=ytc[:],
                              in_=yT.ap().rearrange("(t r) c -> r t c", t=15))
            nc.sync.dma_start(out=wfc[:],
                              in_=wflat.ap().rearrange("(t r) c -> r t c", t=15))

            x1c = sbx.tile([128, 3 * B], f16, name="x1c")
            x1 = [x1c[:, 0:B], x1c[:, B:2 * B], x1c[0:45, 2 * B:3 * B]]
            for dt, (d0, d1) in enumerate(DT):
                dk = d1 - d0
                pp = psa.tile([dk, B], f32, tag="pp", name="pp")
                for t in range(15):
                    nc.tensor.matmul(
                        pp[:],
                        lhsT=wfc[:, t, d0:d1],
                        rhs=ytc[:, t, :],
                        start=(t == 0), stop=(t == 14))
                nc.scalar.activation(x1[dt][0:dk, :], pp[:], AF.Relu)
            # compute ops need quadrant-aligned partition starts; DMA doesn't
            nc.sync.dma_start(out=x1c[44:45, 2 * B:3 * B], in_=ones_d[:])

            # ------------- stage B: head matmuls, copies, exp sums -----------
            logits = sbl.tile([128, NMT * W_SH], f16, name="logits")
            spart = sba.tile([128, NMT * 8], f32, name="spart")
            spack = sba.tile([128, 8], f32, name="spack")     # [glob x4 | sense x4]
            lse = sba.tile([128, 8], f32, name="lse")

            cc_in_g = dram.tile([128, 4], f32, name="cc_in_g")
            cc_out_g = dram.tile([128, 4], f32, addr_space="Shared", name="cc_out_g")
            cc_in_s = dram.tile([128, 4], f32, name="cc_in_s")
            cc_out_s = dram.tile([128, 4], f32, addr_space="Shared", name="cc_out_s")

            whsrc = [wh_r0, wh_r1, wh_r2]

            def emit_exp(mt, ri, c0, c1):
                scratch = sba.tile([128, 1800], f16, tag="exps", name="exps",
                                   bufs=2)
                nc.scalar.activation(
                    scratch[:, 0:c1 - c0],
                    logits[:, mt * W_SH + c0:mt * W_SH + c1], AF.Exp,
                    accum_out=spart[:, mt * 8 + ri:mt * 8 + ri + 1])

            def emit_tail(head, ranges, cc_in, cc_out, lcol):
                # pack sums, AllReduce, lse = Ln(S); then subtract + store
                nr = len(ranges)
                r0 = 0 if head == 0 else len(RANGES_G)
                for mt in range(NMT):
                    nc.vector.reduce_sum(
                        spack[:, lcol + mt:lcol + mt + 1],
                        spart[:, mt * 8 + r0:mt * 8 + r0 + nr],
                        axis=mybir.AxisListType.X)
                nc.sync.dma_start(out=cc_in[:], in_=spack[:, lcol:lcol + 4])
                nc.gpsimd.collective_compute(
                    "AllReduce", mybir.AluOpType.add,
                    replica_groups=[list(range(NCORES))],
                    ins=[cc_in.opt()], outs=[cc_out.opt()])
                sg = sba.tile([128, 4], f32, tag="sg", name="sg", bufs=1)
                nc.sync.dma_start(out=sg[:], in_=cc_out[:])
                nc.scalar.activation(lse[:, lcol:lcol + 4], sg[:], AF.Ln)
                for mt in range(NMT):
                    for (c0, c1) in ranges:
                        stg = sbo.tile([128, 1800], f16, tag="stg", name="stg")
                        nc.vector.tensor_scalar_sub(
                            stg[:, 0:c1 - c0],
                            logits[:, mt * W_SH + c0:mt * W_SH + c1],
                            lse[:, lcol + mt:lcol + mt + 1])
                        nc.gpsimd.dma_start(
                            out=out[128 * mt:128 * (mt + 1), c0:c1],
                            in_=stg[:, 0:c1 - c0])

            for grp in range(NGRP):
                whs = []
                for i, srcArr in enumerate(whsrc):
                    p = 45 if i == 2 else 128
                    wh = sbw.tile([p, GSZ, NTILE], f16, tag=f"wh{i}",
                                  name=f"wh{i}")
                    nc.sync.dma_start(
                        out=wh[:],
                        in_=srcArr[GSZ * grp:GSZ * (grp + 1)].rearrange(
                            "t r c -> r t c"))
                    whs.append(wh)
                for j in range(GSZ):
                    nt = GSZ * grp + j
                    for mt in range(NMT):
                        msl = slice(128 * mt, 128 * (mt + 1))
                        pt = psh.tile([128, NTILE], f32, tag="pt", name="pt")
                        for i in range(3):
                            nc.tensor.matmul(
                                pt[:], lhsT=x1[i][:, msl],
                                rhs=whs[i][:, j, :],
                                start=(i == 0), stop=(i == 2))
                        nc.vector.tensor_copy(
                            logits[:, mt * W_SH + nt * NTILE:
                                   mt * W_SH + (nt + 1) * NTILE], pt[:])
                # exp ranges fully covered by tiles loaded so far
                done = NTILE * GSZ * (grp + 1)
                for ri, (c0, c1) in enumerate(RANGES_G):
                    if done - NTILE * GSZ < c1 <= done:
                        for mt in range(NMT):
                            emit_exp(mt, ri, c0, c1)
                if done >= 6250 and done - NTILE * GSZ < 6250:
                    emit_tail(0, RANGES_G, cc_in_g, cc_out_g, 0)

            for ri, (c0, c1) in enumerate(RANGES_S):
                for mt in range(NMT):
                    emit_exp(mt, len(RANGES_G) + ri, c0, c1)
            emit_tail(1, RANGES_S, cc_in_s, cc_out_s, 4)

    nc.compile()
    return nc


def _host_prep(x, edge_index, edge_type, conv_W, W0, update_gate,
               glob_W, glob_b, sense_W, sense_b, memory0):
    x = np.asarray(x, np.float32)
    ei = np.asarray(edge_index)
    et = np.asarray(edge_type).astype(np.int64)
    src, dst = ei[:, 0, :].astype(np.int64), ei[:, 1, :].astype(np.int64)

    bb = np.broadcast_to(np.arange(B)[:, None], (B, E))
    deg = np.ones((B, N, R), np.float32)
    np.add.at(deg, (bb, dst, et), 1.0)
    dinv = 1.0 / np.sqrt(deg)
    coeff = dinv[bb, src, et] * dinv[bb, dst, et]

    G = np.zeros((B, 6, N), np.float32)
    bm, em = np.nonzero(dst == 0)
    np.add.at(G, (bm, et[bm, em], src[bm, em]), coeff[bm, em])
    G[:, :R, 0] += dinv[:, 0, :] ** 2
    G[:, 5, 0] = 1.0

    Yf = np.einsum("bgn,bnd->bgd", G, x).reshape(B, K)
    Wflat = np.concatenate(
        [np.asarray(conv_W, np.float32).reshape(R * D, D),
         np.asarray(W0, np.float32)], axis=0)

    g = float(np.asarray(update_gate).reshape(-1)[0])
    mem0 = np.asarray(memory0, np.float32)
    if g == 1.0 and not np.any(mem0):
        yT_host = np.ascontiguousarray(Yf.T)
        wf_host = Wflat
    else:
        # exact host fallback for the general carry (linear pre-relu)
        P = Yf @ Wflat
        X1 = np.empty((B, D), np.float32)
        carry = mem0[0].copy()
        for b in range(B):
            carry = g * P[b] + (1.0 - g) * carry
            X1[b] = np.maximum(carry, 0.0)
        yT_host = np.zeros((K, B), np.float32)
        yT_host[:D] = X1.T
        wf_host = np.zeros((K, D), np.float32)
        wf_host[:D] = np.eye(D, dtype=np.float32)

    WcatT = np.concatenate(
        [np.asarray(glob_W, np.float32), np.asarray(sense_W, np.float32)], 0).T
    bcat = np.concatenate(
        [np.asarray(glob_b, np.float32), np.asarray(sense_b, np.float32)], 0)

    wh_r0, wh_r1, wh_r2 = [], [], []
    for c in range(NCORES):
        blk = np.empty((D + 1, W_SH), np.float32)
        g0 = VG_SH * c
        blk[:D, :VG_SH] = WcatT[:, g0:g0 + VG_SH]
        blk[D, :VG_SH] = bcat[g0:g0 + VG_SH]
        s0 = VG + VS_SH * c
        blk[:D, VG_SH:] = WcatT[:, s0:s0 + VS_SH]
        blk[D, VG_SH:] = bcat[s0:s0 + VS_SH]
        tiles = np.ascontiguousarray(
            blk.reshape(D + 1, NNT, NTILE).transpose(1, 0, 2)).astype(np.float16)
        wh_r0.append(np.ascontiguousarray(tiles[:, 0:128]))
        wh_r1.append(np.ascontiguousarray(tiles[:, 128:256]))
        wh_r2.append(np.ascontiguousarray(tiles[:, 256:301]))

    return yT_host.astype(np.float16), wf_host.astype(np.float16), \
        wh_r0, wh_r1, wh_r2


def kernel(**inputs):
    if "nc" not in _CACHE:
        _CACHE["nc"] = _build_device()
    nc = _CACHE["nc"]

    yT_np, wf_np, wh_r0, wh_r1, wh_r2 = _host_prep(**inputs)
    ones_np = np.ones((1, B), np.float16)
    in_maps = [{"yT": yT_np, "wflat": wf_np, "wh_r0": wh_r0[c],
                "wh_r1": wh_r1[c], "wh_r2": wh_r2[c], "ones": ones_np}
               for c in range(NCORES)]

    import os
    trace = bool(int(os.environ.get("KERNEL_TRACE", "0")))
    res = run_bass_kernel_spmd(nc, in_maps, core_ids=list(range(NCORES)),
                               trace=trace)
    _CACHE["last_result"] = res

    outs = [res.results[c]["out"].astype(np.float32) for c in range(NCORES)]
    glob = np.concatenate([o[:, :VG_SH] for o in outs], axis=1)
    sense = np.concatenate([o[:, VG_SH:] for o in outs], axis=1)
    return glob, sense
